# revision 1
# baseline (speedup 1.0000x reference)
"""Trainium2 Bass kernel for causal softmax-free multi-head attention (retention).

Reference computation (per batch b):
    kqv = x @ W1 + b1 ; k, q, v = split(kqv, 3)   [split order k, q, v]
    per head h (dh = 64):  attn = tril(q_h @ k_h^T) ; o_h = attn @ v_h
    out = concat_h(o_h) @ W2 + b2

Sharding: 8 cores = 2 batches x 4 head-groups (4 heads each). Each core
computes its batch's projections restricted to its heads' weight columns,
the attention for its 4 heads, and a partial output projection
(out_local @ W2[rows of its heads]). Host sums the 4 partials per batch.

Algorithm: chunked linear attention. tril(QK^T)V is computed per 256-token
block as  O = Q @ S + tril_block(Q K_blk^T) V_blk, with the running state
S = K^T V accumulated over previous blocks ([64,64] per head). This turns
the O(T^2 dh) dense attention into O(T c dh + T dh^2) work.

Hardware constraints honored (empirically validated on trn2):
  - fp32r matmuls require K=128 contraction, M=128 stationary free dim,
    and N>=256 moving free dim; anything else corrupts results.
    -> dh=64 contractions are zero-padded to 128 rows (kTpad, Spad).
    -> M=64 stationary operands are widened to 128 (head pairs / padding),
       producing garbage rows that are simply never read back.
  - DMA cannot touch PSUM: every matmul result is copied out via DVE/ACT.
  - Producers of fp32r matmul operands must write f32r-typed outputs.
"""

import numpy as np

import concourse.bacc as bacc
import concourse.mybir as mybir
import concourse.tile as tile
from concourse.bass_utils import run_bass_kernel_spmd

F32 = mybir.dt.float32
F32R = mybir.dt.float32r
AF = mybir.ActivationFunctionType

B, T, D = 2, 2048, 1024
H, DH = 16, 64
HPC = 4           # heads per core
FH = HPC * DH     # 256 features per core per tensor
BLK = 256         # state-update block (2 x 128-token chunks)
NBLK = T // BLK   # 8
NTC = T // 128    # 16 token chunks
ND = D // 128     # 8 contraction chunks
NQT = T // 512    # 4 wide token tiles

TRACE = False
TRACE_DIR = None
LAST_RESULTS = [None]


def _build():
    nc = bacc.Bacc("TRN2", target_bir_lowering=False, debug=False, num_devices=8)

    xT = nc.dram_tensor("xT", [D, T], F32, kind="ExternalInput").ap()
    w1q = nc.dram_tensor("w1q", [D, FH], F32, kind="ExternalInput").ap()
    w1kv = nc.dram_tensor("w1kv", [D, 2 * FH], F32, kind="ExternalInput").ap()
    b1q = nc.dram_tensor("b1q", [FH], F32, kind="ExternalInput").ap()
    b1kv = nc.dram_tensor("b1kv", [2 * FH], F32, kind="ExternalInput").ap()
    w2 = nc.dram_tensor("w2", [FH, D], F32, kind="ExternalInput").ap()
    mask0 = nc.dram_tensor("mask0", [128, BLK], F32, kind="ExternalInput").ap()
    mask1 = nc.dram_tensor("mask1", [128, BLK], F32, kind="ExternalInput").ap()
    zer = nc.dram_tensor("zer", [128, T], F32, kind="ExternalInput").ap()
    out = nc.dram_tensor("out", [D, T], F32, kind="ExternalOutput").ap()

    r = lambda ap: ap.bitcast(F32R)

    with tile.TileContext(nc) as tc:
        with (
            tc.tile_pool(name="persist", bufs=1) as pp,
            tc.tile_pool(name="work", bufs=3) as wp,
            tc.tile_pool(name="psA", bufs=4, space="PSUM") as psA,
            tc.tile_pool(name="psO", bufs=2, space="PSUM") as psO,
            tc.tile_pool(name="psU", bufs=2, space="PSUM") as psU,
        ):
            # ---- persistent SBUF tiles -------------------------------------
            w1q_sb = pp.tile([128, ND * FH], F32, name="w1q_sb", tag="w1q_sb")
            w1kv_sb = pp.tile([128, ND * 2 * FH], F32, name="w1kv_sb", tag="w1kv_sb")
            b1q_sb = pp.tile([128, 2], F32, name="b1q_sb", tag="b1q_sb")
            b1k_sb = pp.tile([128, 2], F32, name="b1k_sb", tag="b1k_sb")
            bkv_sb = pp.tile([128, 512], F32, name="bkv_sb", tag="bkv_sb")
            m0_sb = pp.tile([128, BLK], F32, name="m0_sb", tag="m0_sb")
            m1_sb = pp.tile([128, BLK], F32, name="m1_sb", tag="m1_sb")
            qT_sb = [pp.tile([128, T], F32, name=f"qT{g}", tag=f"qT{g}") for g in range(2)]
            kTpad = [pp.tile([128, T], F32, name=f"kTpad{h}", tag=f"kTpad{h}") for h in range(4)]
            kv_sb = [pp.tile([128, 512], F32, name=f"kv{t}", tag=f"kv{t}") for t in range(NTC)]
            oT_sb = [pp.tile([128, T], F32, name=f"oT{g}", tag=f"oT{g}") for g in range(2)]

            with tc.tile_pool(name="xt", bufs=1) as xp:
                xt = [xp.tile([128, T], F32, name=f"xt{i}", tag=f"xt{i}") for i in range(ND)]
                # x^T chunks on gpsimd queues, weight chunks on sync queues —
                # DMA triggers cost ~0.65us each on the issuing engine, so
                # spread them and keep the count low.
                # Chunk-interleaved input stream on one queue: the projection
                # waves below consume chunk d of (x^T, W1q, W1kv) together, so
                # deliver them together and in order.
                nc.sync.dma_start(out=r(m0_sb[:]), in_=r(mask0[:]))
                nc.sync.dma_start(out=r(m1_sb[:]), in_=r(mask1[:]))
                # Column-halved x^T stream: all chunks' first 1024 token-cols
                # land first, so the qt0-1 projection waves (and the first 8
                # KV groups) are fully runnable while the second half streams.
                HT = T // 2
                for i in range(ND):
                    nc.sync.dma_start(out=r(xt[i][:, 0:HT]), in_=r(xT[128 * i:128 * (i + 1), 0:HT]))
                    nc.sync.dma_start(
                        out=r(w1q_sb[:, i * FH:(i + 1) * FH]),
                        in_=r(w1q[128 * i:128 * (i + 1), :]))
                    nc.gpsimd.dma_start(
                        out=r(w1kv_sb[:, i * 2 * FH:(i + 1) * 2 * FH]),
                        in_=r(w1kv[128 * i:128 * (i + 1), :]))
                    if i == 2:
                        nc.gpsimd.dma_start(out=b1q_sb[:], in_=b1q.rearrange("(c p) -> p c", p=128))
                        nc.gpsimd.dma_start(out=b1k_sb[:], in_=b1kv[0:256].rearrange("(c p) -> p c", p=128))
                        nc.gpsimd.dma_start(out=bkv_sb[:], in_=b1kv.unsqueeze(0).broadcast_to([128, 512]))
                for i in range(ND):
                    nc.sync.dma_start(out=r(xt[i][:, HT:T]), in_=r(xT[128 * i:128 * (i + 1), HT:T]))
                # zero fills last — only needed by phase C
                for h in range(4):
                    par = h % 2
                    nc.sync.dma_start(
                        out=r(kTpad[h][(1 - par) * 64:(2 - par) * 64, :]),
                        in_=r(zer[0:64, :]))

                # ---- phase B: projections ----------------------------------
                # Waves of 8 concurrent PSUM groups; within a wave the
                # contraction chunk d is the OUTER loop so the (in-order) PE
                # stream can run each chunk's matmuls as soon as that chunk
                # lands, instead of blocking on the last chunk of group 0.
                _pools = [(psA, "pa"), (psU, "pu"), (psO, "po"),
                          (psA, "pa"), (psU, "pu"), (psO, "po"),
                          (psA, "pa"), (psA, "pa")]

                def run_wave(groups, pools=None):
                    # groups: list of (lhsT_fn(d), rhs_fn(d), copyback_fn)
                    pl = pools if pools is not None else _pools
                    tiles = []
                    for gi, _ in enumerate(groups):
                        pool, tag = pl[gi]
                        tiles.append(pool.tile([128, 512], F32, name=f"pw{gi}", tag=tag))
                    for d in range(ND):
                        for gi, (lf, rf, _) in enumerate(groups):
                            nc.tensor.matmul(
                                tiles[gi][:], lf(d), rf(d),
                                start=(d == 0), stop=(d == ND - 1))
                    for gi, (_, _, cb) in enumerate(groups):
                        cb(tiles[gi])

                def q_group(ft, qt):
                    def cb(pt):
                        nc.scalar.activation(
                            r(qT_sb[ft][:, qt * 512:(qt + 1) * 512]), pt[:],
                            AF.Identity, bias=b1q_sb[:, ft:ft + 1])
                    return (
                        lambda d: r(w1q_sb[:, d * FH + ft * 128: d * FH + (ft + 1) * 128]),
                        lambda d: r(xt[d][:, qt * 512:(qt + 1) * 512]),
                        cb)

                def k_group(ft, qt):
                    def cb(pt):
                        for par in range(2):
                            h = 2 * ft + par
                            sl = slice(par * 64, (par + 1) * 64)
                            nc.scalar.activation(
                                r(kTpad[h][sl, qt * 512:(qt + 1) * 512]), pt[sl, :],
                                AF.Identity, bias=b1k_sb[sl, ft:ft + 1])
                    return (
                        lambda d: r(w1kv_sb[:, d * 2 * FH + ft * 128: d * 2 * FH + (ft + 1) * 128]),
                        lambda d: r(xt[d][:, qt * 512:(qt + 1) * 512]),
                        cb)

                def kv_group(tcn):
                    def cb(pt):
                        nc.vector.tensor_tensor(
                            r(kv_sb[tcn][:]), pt[:], bkv_sb[:], mybir.AluOpType.add)
                    return (
                        lambda d: r(xt[d][:, tcn * 128:(tcn + 1) * 128]),
                        lambda d: r(w1kv_sb[:, d * 2 * FH:(d + 1) * 2 * FH]),
                        cb)

                # Wave order matched to the half-column stream: W1 (qt0-1
                # of Q^T/K^T) runs during the first half, W3 (KV tcn0-7, all
                # first-half data) keeps the PE saturated while the second
                # half streams, then W2 (qt2-3) and W4 (KV tcn8-15).
                run_wave([q_group(0, 0), q_group(0, 1), k_group(0, 0), k_group(0, 1),
                          q_group(1, 0), q_group(1, 1), k_group(1, 0), k_group(1, 1)])
                run_wave([kv_group(t) for t in range(8)])
                run_wave([q_group(0, 2), q_group(0, 3), k_group(0, 2), k_group(0, 3),
                          q_group(1, 2), q_group(1, 3), k_group(1, 2), k_group(1, 3)])
                run_wave([kv_group(t) for t in range(8, 16)])

            # ---- late pool: state tiles + W2 (reuses x^T space) ------------
            with tc.tile_pool(name="late", bufs=1) as lp:
                spad = [lp.tile([128, 128], F32, name=f"spad{h}", tag=f"spad{h}") for h in range(4)]
                w2_sb = lp.tile([128, 2 * D], F32, name="w2_sb", tag="w2_sb")
                # manually-rotated a1 ring: the left half of each slot is the
                # always-zero region of the chunk-1 scores; zero it once via
                # DMA and let the per-block mask multiply touch only the tril
                # half. Tile tracks WAR deps on the persistent tiles.
                a1ring = [lp.tile([128, 2 * BLK], F32, name=f"a1r{i}", tag=f"a1r{i}")
                          for i in range(4)]
                for i in range(4):
                    for par in range(2):
                        nc.gpsimd.dma_start(
                            out=r(a1ring[i][:, par * BLK: par * BLK + 128]),
                            in_=r(zer[:, 0:128]))
                for h in range(4):
                    nc.gpsimd.dma_start(out=r(spad[h][:]), in_=r(zer[:, 0:128]))
                nc.sync.dma_start(
                    out=r(w2_sb.rearrange("p (c f) -> p c f", c=2)),
                    in_=r(w2.rearrange("(c p) f -> p c f", p=128)))

                # ---- phase C: chunked causal attention + interleaved D -----
                # Two-stage software pipeline: block m's scores are emitted
                # before block m-1's O-accumulation chains, so the in-order PE
                # stream always has independent matmuls while the DVE applies
                # causal masks for the previous block.
                ablk = {}

                def scores_block(m):
                    qsl = slice(m * BLK, (m + 1) * BLK)
                    for pg in range(2):
                        a0 = lp.tile([128, 2 * BLK], F32, name="a0", tag="a0", bufs=4)
                        a1 = a1ring[(2 * m + pg) % 4]
                        ablk[(m, pg)] = (a0, a1)
                        for par in range(2):
                            h = 2 * pg + par
                            asl = slice(par * BLK, (par + 1) * BLK)
                            pA = psA.tile([128, 2 * BLK], F32, name="pA", tag="pa")
                            nc.tensor.matmul(
                                pA[:, 0:BLK], r(kTpad[h][:, (2 * m) * 128:(2 * m + 1) * 128]),
                                r(qT_sb[pg][:, qsl]), start=True, stop=True)
                            nc.tensor.matmul(
                                pA[:, BLK:2 * BLK], r(kTpad[h][:, (2 * m + 1) * 128:(2 * m + 2) * 128]),
                                r(qT_sb[pg][:, qsl]), start=True, stop=True, skip_group_check=True)
                            nc.vector.tensor_tensor(r(a0[:, asl]), pA[:, 0:BLK], m0_sb[:], mybir.AluOpType.mult)
                            # only the tril half: the left 128 cols stay zero
                            nc.vector.tensor_tensor(
                                r(a1[:, par * BLK + 128: (par + 1) * BLK]),
                                pA[:, BLK + 128:2 * BLK], m1_sb[:, 128:BLK],
                                mybir.AluOpType.mult)

                def chains_block(m):
                    qsl = slice(m * BLK, (m + 1) * BLK)
                    for pg in range(2):
                        a0, a1 = ablk.pop((m, pg))
                        pO = psO.tile([128, 2 * BLK], F32, name="pO", tag="po")
                        nc.tensor.matmul(
                            pO[:], r(kv_sb[2 * m][:, FH + pg * 128: FH + (pg + 1) * 128]),
                            r(a0[:]), start=True, stop=False)
                        nc.tensor.matmul(
                            pO[:], r(kv_sb[2 * m + 1][:, FH + pg * 128: FH + (pg + 1) * 128]),
                            r(a1[:]), start=False, stop=(m == 0))
                        if m > 0:
                            nc.tensor.matmul(
                                pO[:, 0:BLK], r(spad[2 * pg][:]), r(qT_sb[pg][:, qsl]),
                                start=False, stop=False)
                            nc.tensor.matmul(
                                pO[:, BLK:2 * BLK], r(spad[2 * pg + 1][:]), r(qT_sb[pg][:, qsl]),
                                start=False, stop=True)
                        for par in range(2):
                            hr = slice(par * 64, (par + 1) * 64)
                            nc.scalar.activation(
                                r(oT_sb[pg][hr, qsl]),
                                pO[hr, par * BLK:par * BLK + BLK], AF.Identity)

                    for pg in range(2):
                        pU = psU.tile([128, BLK], F32, name="pU", tag="pu")
                        nc.tensor.matmul(
                            pU[:], r(kv_sb[2 * m][:, pg * 128:(pg + 1) * 128]),
                            r(kv_sb[2 * m][:, FH:2 * FH]), start=True, stop=False)
                        nc.tensor.matmul(
                            pU[:], r(kv_sb[2 * m + 1][:, pg * 128:(pg + 1) * 128]),
                            r(kv_sb[2 * m + 1][:, FH:2 * FH]), start=False, stop=True)
                        for par in range(2):
                            h = 2 * pg + par
                            hr = slice(par * 64, (par + 1) * 64)
                            nc.vector.tensor_tensor(
                                r(spad[h][hr, hr]), pU[hr, h * 64:(h + 1) * 64],
                                spad[h][hr, hr], mybir.AluOpType.add)

                def proj_tile(qt, half):
                    dcr = range(0, ND // 2) if half == 0 else range(ND // 2, ND)
                    for dc in dcr:
                        pf = [psA.tile([128, 512], F32, name="pf", tag="pa"),
                              psU.tile([128, 512], F32, name="pf2", tag="pu"),
                              psO.tile([128, 512], F32, name="pf3", tag="po")][dc % 3]
                        for g2 in range(2):
                            nc.tensor.matmul(
                                pf[:],
                                r(w2_sb[:, g2 * D + dc * 128: g2 * D + (dc + 1) * 128]),
                                r(oT_sb[g2][:, qt * 512:(qt + 1) * 512]),
                                start=(g2 == 0), stop=(g2 == 1))
                        fs = lp.tile([128, 512], F32, name="fs", tag="fs", bufs=3)
                        if dc % 2 == 0:
                            nc.vector.tensor_copy(fs[:], pf[:])
                        else:
                            nc.scalar.activation(fs[:], pf[:], AF.Identity)
                        dma_eng = nc.gpsimd if dc % 2 == 0 else nc.sync
                        dma_eng.dma_start(
                            out=out[dc * 128:(dc + 1) * 128, qt * 512:(qt + 1) * 512],
                            in_=fs[:])

                def proj_tile256(tcn):
                    for dc in range(ND):
                        pf = psA.tile([128, 2 * BLK], F32, name="pf3", tag="pa") if dc % 2 == 0 \
                            else psU.tile([128, 2 * BLK], F32, name="pf4", tag="pu")
                        for g2 in range(2):
                            nc.tensor.matmul(
                                pf[:, 0:BLK],
                                r(w2_sb[:, g2 * D + dc * 128: g2 * D + (dc + 1) * 128]),
                                r(oT_sb[g2][:, tcn * BLK:(tcn + 1) * BLK]),
                                start=(g2 == 0), stop=(g2 == 1))
                        fs = lp.tile([128, BLK], F32, name="fs2", tag="fs2", bufs=3)
                        if dc % 2 == 0:
                            nc.vector.tensor_copy(fs[:], pf[:, 0:BLK])
                        else:
                            nc.scalar.activation(fs[:], pf[:, 0:BLK], AF.Identity)
                        dma_eng = nc.gpsimd if dc % 2 == 0 else nc.sync
                        dma_eng.dma_start(
                            out=out[dc * 128:(dc + 1) * 128, tcn * BLK:(tcn + 1) * BLK],
                            in_=fs[:])

                # proj_tile(qt) is emitted one full block after the ACT
                # copybacks that produce its oT inputs, so the PE stream never
                # stalls waiting for the Scalar engine to catch up.
                # D tiles are spread as half-emissions (4 dout chunks each)
                # across blocks, one-plus blocks after the chains that produce
                # their oT inputs.
                dplan = {3: (0, 0), 4: (0, 1), 5: (1, 0), 6: (1, 1), 7: (2, 0)}
                scores_block(0)
                for m in range(1, NBLK):
                    scores_block(m)
                    chains_block(m - 1)
                    if m in dplan:
                        proj_tile(*dplan[m])
                chains_block(NBLK - 1)
                proj_tile(2, 1)
                proj_tile(3, 0)
                proj_tile(3, 1)

    nc.compile()
    return nc


_NC = None


def _get_nc():
    global _NC
    if _NC is None:
        _NC = _build()
    return _NC


def make_core_inputs(x, W1, b1, W2, b2):
    """Shard full inputs into the 8 per-core input dicts."""
    x = np.asarray(x, dtype=np.float32)
    W1 = np.asarray(W1, dtype=np.float32)
    b1 = np.asarray(b1, dtype=np.float32)
    W2 = np.asarray(W2, dtype=np.float32)

    p = np.arange(128)[:, None]
    f = np.arange(BLK)[None, :]
    mask0 = (f >= p).astype(np.float32)
    mask1 = (f >= p + 128).astype(np.float32)
    zer = np.zeros((128, T), np.float32)

    in_maps = []
    for c in range(8):
        b = c // 4
        g = c % 4
        ksl = slice(g * FH, (g + 1) * FH)
        qsl = slice(D + g * FH, D + (g + 1) * FH)
        vsl = slice(2 * D + g * FH, 2 * D + (g + 1) * FH)
        in_maps.append({
            "xT": np.ascontiguousarray(x[b].T),
            "w1q": np.ascontiguousarray(W1[:, qsl]),
            "w1kv": np.ascontiguousarray(np.concatenate([W1[:, ksl], W1[:, vsl]], axis=1)),
            "b1q": np.ascontiguousarray(b1[qsl]),
            "b1kv": np.ascontiguousarray(np.concatenate([b1[ksl], b1[vsl]])),
            "w2": np.ascontiguousarray(W2[ksl, :]),
            "mask0": mask0,
            "mask1": mask1,
            "zer": zer,
        })
    return in_maps


def kernel(x, W1, b1, W2, b2):
    nc = _get_nc()
    in_maps = make_core_inputs(x, W1, b1, W2, b2)
    kwargs = {}
    if TRACE:
        kwargs = {"trace": True, "tmpdir": TRACE_DIR}
    res = run_bass_kernel_spmd(nc, in_maps, list(range(8)), **kwargs)
    LAST_RESULTS[0] = res
    b2 = np.asarray(b2, dtype=np.float32)
    out = np.zeros((B, T, D), np.float32)
    for c in range(8):
        out[c // 4] += res.results[c]["out"].T
    out += b2[None, None, :]
    return out



# revision 6
# speedup vs baseline: 1.1235x; 1.1235x over previous
"""Trainium2 Bass kernel for causal softmax-free multi-head attention (retention).

Reference computation (per batch b):
    kqv = x @ W1 + b1 ; k, q, v = split(kqv, 3)   [split order k, q, v]
    per head h (dh = 64):  attn = tril(q_h @ k_h^T) ; o_h = attn @ v_h
    out = concat_h(o_h) @ W2 + b2

Sharding: 8 cores = 2 batches x 4 head-groups (4 heads each). Each core
computes its batch's projections restricted to its heads' weight columns,
the attention for its 4 heads, and a partial output projection
(out_local @ W2[rows of its heads]). Host sums the 4 partials per batch.

Algorithm: chunked linear attention. tril(QK^T)V is computed per 256-token
block as  O = Q @ S + tril_block(Q K_blk^T) V_blk, with the running state
S = K^T V accumulated over previous blocks ([64,64] per head). This turns
the O(T^2 dh) dense attention into O(T c dh + T dh^2) work.

Hardware constraints honored (empirically validated on trn2):
  - fp32r matmuls require K=128 contraction, M=128 stationary free dim,
    and N>=256 moving free dim; anything else corrupts results.
    -> dh=64 contractions are zero-padded to 128 rows (kTpad, Spad).
    -> M=64 stationary operands are widened to 128 (head pairs / padding),
       producing garbage rows that are simply never read back.
  - DMA cannot touch PSUM: every matmul result is copied out via DVE/ACT.
  - Producers of fp32r matmul operands must write f32r-typed outputs.
"""

import numpy as np

import concourse.bacc as bacc
import concourse.mybir as mybir
import concourse.tile as tile
from concourse.bass_utils import run_bass_kernel_spmd

F32 = mybir.dt.float32
F32R = mybir.dt.float32r
BF16 = mybir.dt.bfloat16
AF = mybir.ActivationFunctionType

B, T, D = 2, 2048, 1024
H, DH = 16, 64
HPC = 4           # heads per core
FH = HPC * DH     # 256 features per core per tensor
BLK = 256         # state-update block (2 x 128-token chunks)
NBLK = T // BLK   # 8
NTC = T // 128    # 16 token chunks
ND = D // 128     # 8 contraction chunks
NQT = T // 512    # 4 wide token tiles

TRACE = False
TRACE_DIR = None
LAST_RESULTS = [None]


def _build():
    nc = bacc.Bacc("TRN2", target_bir_lowering=False, debug=False, num_devices=8)

    xT = nc.dram_tensor("xT", [D, T], BF16, kind="ExternalInput").ap()
    w1q = nc.dram_tensor("w1q", [D, FH], BF16, kind="ExternalInput").ap()
    w1kv = nc.dram_tensor("w1kv", [D, 2 * FH], BF16, kind="ExternalInput").ap()
    b1q = nc.dram_tensor("b1q", [FH], F32, kind="ExternalInput").ap()
    b1kv = nc.dram_tensor("b1kv", [2 * FH], F32, kind="ExternalInput").ap()
    w2 = nc.dram_tensor("w2", [FH, D], BF16, kind="ExternalInput").ap()
    mask0 = nc.dram_tensor("mask0", [128, BLK], F32, kind="ExternalInput").ap()
    mask1 = nc.dram_tensor("mask1", [128, BLK], F32, kind="ExternalInput").ap()
    zer = nc.dram_tensor("zer", [128, T], BF16, kind="ExternalInput").ap()
    out = nc.dram_tensor("out", [D, T], BF16, kind="ExternalOutput").ap()

    r = lambda ap: ap

    with tile.TileContext(nc) as tc:
        with (
            tc.tile_pool(name="persist", bufs=1) as pp,
            tc.tile_pool(name="work", bufs=3) as wp,
            tc.tile_pool(name="psA", bufs=4, space="PSUM") as psA,
            tc.tile_pool(name="psO", bufs=2, space="PSUM") as psO,
            tc.tile_pool(name="psU", bufs=2, space="PSUM") as psU,
        ):
            # ---- persistent SBUF tiles -------------------------------------
            w1q_sb = pp.tile([128, ND * FH], BF16, name="w1q_sb", tag="w1q_sb")
            w1kv_sb = pp.tile([128, ND * 2 * FH], BF16, name="w1kv_sb", tag="w1kv_sb")
            b1q_sb = pp.tile([128, 2], F32, name="b1q_sb", tag="b1q_sb")
            b1k_sb = pp.tile([128, 2], F32, name="b1k_sb", tag="b1k_sb")
            bkv_sb = pp.tile([128, 512], F32, name="bkv_sb", tag="bkv_sb")
            m0_sb = pp.tile([128, BLK], F32, name="m0_sb", tag="m0_sb")
            m1_sb = pp.tile([128, BLK], F32, name="m1_sb", tag="m1_sb")
            qT_sb = [pp.tile([128, T], BF16, name=f"qT{g}", tag=f"qT{g}") for g in range(2)]
            kTpad = [pp.tile([128, T], BF16, name=f"kTpad{h}", tag=f"kTpad{h}") for h in range(4)]
            kv_sb = [pp.tile([128, 512], BF16, name=f"kv{t}", tag=f"kv{t}") for t in range(NTC)]
            oT_sb = [pp.tile([128, T], BF16, name=f"oT{g}", tag=f"oT{g}") for g in range(2)]

            with tc.tile_pool(name="xt", bufs=1) as xp:
                xt = [xp.tile([128, T], BF16, name=f"xt{i}", tag=f"xt{i}") for i in range(ND)]
                # x^T chunks on gpsimd queues, weight chunks on sync queues —
                # DMA triggers cost ~0.65us each on the issuing engine, so
                # spread them and keep the count low.
                # Chunk-interleaved input stream on one queue: the projection
                # waves below consume chunk d of (x^T, W1q, W1kv) together, so
                # deliver them together and in order.
                nc.sync.dma_start(out=r(m0_sb[:]), in_=r(mask0[:]))
                nc.sync.dma_start(out=r(m1_sb[:]), in_=r(mask1[:]))
                # Column-halved x^T stream: all chunks' first 1024 token-cols
                # land first, so the qt0-1 projection waves (and the first 8
                # KV groups) are fully runnable while the second half streams.
                HT = T // 2
                for i in range(ND):
                    nc.sync.dma_start(out=r(xt[i][:, 0:HT]), in_=r(xT[128 * i:128 * (i + 1), 0:HT]))
                    nc.sync.dma_start(
                        out=r(w1q_sb[:, i * FH:(i + 1) * FH]),
                        in_=r(w1q[128 * i:128 * (i + 1), :]))
                    nc.gpsimd.dma_start(
                        out=r(w1kv_sb[:, i * 2 * FH:(i + 1) * 2 * FH]),
                        in_=r(w1kv[128 * i:128 * (i + 1), :]))
                    if i == 2:
                        nc.gpsimd.dma_start(out=b1q_sb[:], in_=b1q.rearrange("(c p) -> p c", p=128))
                        nc.gpsimd.dma_start(out=b1k_sb[:], in_=b1kv[0:256].rearrange("(c p) -> p c", p=128))
                        nc.gpsimd.dma_start(out=bkv_sb[:], in_=b1kv.unsqueeze(0).broadcast_to([128, 512]))
                for i in range(ND):
                    nc.sync.dma_start(out=r(xt[i][:, HT:T]), in_=r(xT[128 * i:128 * (i + 1), HT:T]))
                # zero fills last — only needed by phase C
                for h in range(4):
                    par = h % 2
                    nc.sync.dma_start(
                        out=r(kTpad[h][(1 - par) * 64:(2 - par) * 64, :]),
                        in_=r(zer[0:64, :]))

                # ---- phase B: projections ----------------------------------
                # Waves of 8 concurrent PSUM groups; within a wave the
                # contraction chunk d is the OUTER loop so the (in-order) PE
                # stream can run each chunk's matmuls as soon as that chunk
                # lands, instead of blocking on the last chunk of group 0.
                _pools = [(psA, "pa"), (psU, "pu"), (psO, "po"),
                          (psA, "pa"), (psU, "pu"), (psO, "po"),
                          (psA, "pa"), (psA, "pa")]

                def run_wave(groups, pools=None):
                    # groups: list of (lhsT_fn(d), rhs_fn(d), copyback_fn)
                    pl = pools if pools is not None else _pools
                    tiles = []
                    for gi, _ in enumerate(groups):
                        pool, tag = pl[gi]
                        tiles.append(pool.tile([128, 512], F32, name=f"pw{gi}", tag=tag))
                    for d in range(ND):
                        for gi, (lf, rf, _) in enumerate(groups):
                            nc.tensor.matmul(
                                tiles[gi][:], lf(d), rf(d),
                                start=(d == 0), stop=(d == ND - 1))
                    for gi, (_, _, cb) in enumerate(groups):
                        cb(tiles[gi])

                def q_group(ft, qt):
                    def cb(pt):
                        nc.scalar.activation(
                            r(qT_sb[ft][:, qt * 512:(qt + 1) * 512]), pt[:],
                            AF.Identity, bias=b1q_sb[:, ft:ft + 1])
                    return (
                        lambda d: r(w1q_sb[:, d * FH + ft * 128: d * FH + (ft + 1) * 128]),
                        lambda d: r(xt[d][:, qt * 512:(qt + 1) * 512]),
                        cb)

                def k_group(ft, qt):
                    def cb(pt):
                        for par in range(2):
                            h = 2 * ft + par
                            sl = slice(par * 64, (par + 1) * 64)
                            nc.scalar.activation(
                                r(kTpad[h][sl, qt * 512:(qt + 1) * 512]), pt[sl, :],
                                AF.Identity, bias=b1k_sb[sl, ft:ft + 1])
                    return (
                        lambda d: r(w1kv_sb[:, d * 2 * FH + ft * 128: d * 2 * FH + (ft + 1) * 128]),
                        lambda d: r(xt[d][:, qt * 512:(qt + 1) * 512]),
                        cb)

                def kv_group(tcn):
                    def cb(pt):
                        nc.vector.tensor_tensor(
                            r(kv_sb[tcn][:]), pt[:], bkv_sb[:], mybir.AluOpType.add)
                    return (
                        lambda d: r(xt[d][:, tcn * 128:(tcn + 1) * 128]),
                        lambda d: r(w1kv_sb[:, d * 2 * FH:(d + 1) * 2 * FH]),
                        cb)

                # Wave order matched to the half-column stream: W1 (qt0-1
                # of Q^T/K^T) runs during the first half, W3 (KV tcn0-7, all
                # first-half data) keeps the PE saturated while the second
                # half streams, then W2 (qt2-3) and W4 (KV tcn8-15).
                run_wave([q_group(0, 0), q_group(0, 1), k_group(0, 0), k_group(0, 1),
                          q_group(1, 0), q_group(1, 1), k_group(1, 0), k_group(1, 1)])
                run_wave([kv_group(t) for t in range(8)])
                run_wave([q_group(0, 2), q_group(0, 3), k_group(0, 2), k_group(0, 3),
                          q_group(1, 2), q_group(1, 3), k_group(1, 2), k_group(1, 3)])
                run_wave([kv_group(t) for t in range(8, 16)])

            # ---- late pool: state tiles + W2 (reuses x^T space) ------------
            with tc.tile_pool(name="late", bufs=1) as lp:
                spad = [lp.tile([128, 128], BF16, name=f"spad{h}", tag=f"spad{h}") for h in range(4)]
                w2_sb = lp.tile([128, 2 * D], BF16, name="w2_sb", tag="w2_sb")
                # manually-rotated a1 ring: the left half of each slot is the
                # always-zero region of the chunk-1 scores; zero it once via
                # DMA and let the per-block mask multiply touch only the tril
                # half. Tile tracks WAR deps on the persistent tiles.
                a1ring = [lp.tile([128, 2 * BLK], BF16, name=f"a1r{i}", tag=f"a1r{i}")
                          for i in range(4)]
                for i in range(4):
                    for par in range(2):
                        nc.gpsimd.dma_start(
                            out=r(a1ring[i][:, par * BLK: par * BLK + 128]),
                            in_=r(zer[:, 0:128]))
                for h in range(4):
                    nc.gpsimd.dma_start(out=r(spad[h][:]), in_=r(zer[:, 0:128]))
                nc.sync.dma_start(
                    out=r(w2_sb.rearrange("p (c f) -> p c f", c=2)),
                    in_=r(w2.rearrange("(c p) f -> p c f", p=128)))

                # ---- phase C: chunked causal attention + interleaved D -----
                # Two-stage software pipeline: block m's scores are emitted
                # before block m-1's O-accumulation chains, so the in-order PE
                # stream always has independent matmuls while the DVE applies
                # causal masks for the previous block.
                ablk = {}

                def scores_block(m):
                    qsl = slice(m * BLK, (m + 1) * BLK)
                    for pg in range(2):
                        a0 = lp.tile([128, 2 * BLK], BF16, name="a0", tag="a0", bufs=4)
                        a1 = a1ring[(2 * m + pg) % 4]
                        ablk[(m, pg)] = (a0, a1)
                        for par in range(2):
                            h = 2 * pg + par
                            asl = slice(par * BLK, (par + 1) * BLK)
                            pA = psA.tile([128, 2 * BLK], F32, name="pA", tag="pa")
                            nc.tensor.matmul(
                                pA[:, 0:BLK], r(kTpad[h][:, (2 * m) * 128:(2 * m + 1) * 128]),
                                r(qT_sb[pg][:, qsl]), start=True, stop=True)
                            nc.tensor.matmul(
                                pA[:, BLK:2 * BLK], r(kTpad[h][:, (2 * m + 1) * 128:(2 * m + 2) * 128]),
                                r(qT_sb[pg][:, qsl]), start=True, stop=True, skip_group_check=True)
                            nc.vector.tensor_tensor(r(a0[:, asl]), pA[:, 0:BLK], m0_sb[:], mybir.AluOpType.mult)
                            # only the tril half: the left 128 cols stay zero
                            nc.vector.tensor_tensor(
                                r(a1[:, par * BLK + 128: (par + 1) * BLK]),
                                pA[:, BLK + 128:2 * BLK], m1_sb[:, 128:BLK],
                                mybir.AluOpType.mult)

                def chains_block(m):
                    qsl = slice(m * BLK, (m + 1) * BLK)
                    for pg in range(2):
                        a0, a1 = ablk.pop((m, pg))
                        pO = psO.tile([128, 2 * BLK], F32, name="pO", tag="po")
                        nc.tensor.matmul(
                            pO[:], r(kv_sb[2 * m][:, FH + pg * 128: FH + (pg + 1) * 128]),
                            r(a0[:]), start=True, stop=False)
                        nc.tensor.matmul(
                            pO[:], r(kv_sb[2 * m + 1][:, FH + pg * 128: FH + (pg + 1) * 128]),
                            r(a1[:]), start=False, stop=(m == 0))
                        if m > 0:
                            nc.tensor.matmul(
                                pO[:, 0:BLK], r(spad[2 * pg][:]), r(qT_sb[pg][:, qsl]),
                                start=False, stop=False)
                            nc.tensor.matmul(
                                pO[:, BLK:2 * BLK], r(spad[2 * pg + 1][:]), r(qT_sb[pg][:, qsl]),
                                start=False, stop=True)
                        for par in range(2):
                            hr = slice(par * 64, (par + 1) * 64)
                            nc.scalar.activation(
                                r(oT_sb[pg][hr, qsl]),
                                pO[hr, par * BLK:par * BLK + BLK], AF.Identity)

                    for pg in range(2):
                        pU = psU.tile([128, BLK], F32, name="pU", tag="pu")
                        nc.tensor.matmul(
                            pU[:], r(kv_sb[2 * m][:, pg * 128:(pg + 1) * 128]),
                            r(kv_sb[2 * m][:, FH:2 * FH]), start=True, stop=False)
                        nc.tensor.matmul(
                            pU[:], r(kv_sb[2 * m + 1][:, pg * 128:(pg + 1) * 128]),
                            r(kv_sb[2 * m + 1][:, FH:2 * FH]), start=False, stop=True)
                        for par in range(2):
                            h = 2 * pg + par
                            hr = slice(par * 64, (par + 1) * 64)
                            nc.vector.tensor_tensor(
                                r(spad[h][hr, hr]), pU[hr, h * 64:(h + 1) * 64],
                                spad[h][hr, hr], mybir.AluOpType.add)

                def proj_tile(qt, half):
                    dcr = range(0, ND // 2) if half == 0 else range(ND // 2, ND)
                    for dc in dcr:
                        pf = [psA.tile([128, 512], F32, name="pf", tag="pa"),
                              psU.tile([128, 512], F32, name="pf2", tag="pu"),
                              psO.tile([128, 512], F32, name="pf3", tag="po")][dc % 3]
                        for g2 in range(2):
                            nc.tensor.matmul(
                                pf[:],
                                r(w2_sb[:, g2 * D + dc * 128: g2 * D + (dc + 1) * 128]),
                                r(oT_sb[g2][:, qt * 512:(qt + 1) * 512]),
                                start=(g2 == 0), stop=(g2 == 1))
                        fs = lp.tile([128, 512], BF16, name="fs", tag="fs", bufs=3)
                        if dc % 2 == 0:
                            nc.vector.tensor_copy(fs[:], pf[:])
                        else:
                            nc.scalar.activation(fs[:], pf[:], AF.Identity)
                        dma_eng = nc.gpsimd if dc % 2 == 0 else nc.sync
                        dma_eng.dma_start(
                            out=out[dc * 128:(dc + 1) * 128, qt * 512:(qt + 1) * 512],
                            in_=fs[:])

                def proj_tile256(tcn):
                    for dc in range(ND):
                        pf = psA.tile([128, 2 * BLK], F32, name="pf3", tag="pa") if dc % 2 == 0 \
                            else psU.tile([128, 2 * BLK], F32, name="pf4", tag="pu")
                        for g2 in range(2):
                            nc.tensor.matmul(
                                pf[:, 0:BLK],
                                r(w2_sb[:, g2 * D + dc * 128: g2 * D + (dc + 1) * 128]),
                                r(oT_sb[g2][:, tcn * BLK:(tcn + 1) * BLK]),
                                start=(g2 == 0), stop=(g2 == 1))
                        fs = lp.tile([128, BLK], BF16, name="fs2", tag="fs2", bufs=3)
                        if dc % 2 == 0:
                            nc.vector.tensor_copy(fs[:], pf[:, 0:BLK])
                        else:
                            nc.scalar.activation(fs[:], pf[:, 0:BLK], AF.Identity)
                        dma_eng = nc.gpsimd if dc % 2 == 0 else nc.sync
                        dma_eng.dma_start(
                            out=out[dc * 128:(dc + 1) * 128, tcn * BLK:(tcn + 1) * BLK],
                            in_=fs[:])

                # proj_tile(qt) is emitted one full block after the ACT
                # copybacks that produce its oT inputs, so the PE stream never
                # stalls waiting for the Scalar engine to catch up.
                # D tiles are spread as half-emissions (4 dout chunks each)
                # across blocks, one-plus blocks after the chains that produce
                # their oT inputs.
                dplan = {3: (0, 0), 4: (0, 1), 5: (1, 0), 6: (1, 1), 7: (2, 0)}
                scores_block(0)
                for m in range(1, NBLK):
                    scores_block(m)
                    chains_block(m - 1)
                    if m in dplan:
                        proj_tile(*dplan[m])
                chains_block(NBLK - 1)
                proj_tile(2, 1)
                proj_tile(3, 0)
                proj_tile(3, 1)

    nc.compile()
    return nc


_NC = None


def _get_nc():
    global _NC
    if _NC is None:
        _NC = _build()
    return _NC


def make_core_inputs(x, W1, b1, W2, b2):
    """Shard full inputs into the 8 per-core input dicts."""
    import ml_dtypes
    BF = ml_dtypes.bfloat16
    x = np.asarray(x, dtype=np.float32)
    W1 = np.asarray(W1, dtype=np.float32).astype(BF)
    b1 = np.asarray(b1, dtype=np.float32)
    W2 = np.asarray(W2, dtype=np.float32)

    p = np.arange(128)[:, None]
    f = np.arange(BLK)[None, :]
    mask0 = (f >= p).astype(np.float32)
    mask1 = (f >= p + 128).astype(np.float32)
    zer = np.zeros((128, T), BF)

    in_maps = []
    for c in range(8):
        b = c // 4
        g = c % 4
        ksl = slice(g * FH, (g + 1) * FH)
        qsl = slice(D + g * FH, D + (g + 1) * FH)
        vsl = slice(2 * D + g * FH, 2 * D + (g + 1) * FH)
        in_maps.append({
            "xT": np.ascontiguousarray(x[b].T.astype(BF)),
            "w1q": np.ascontiguousarray(W1[:, qsl]),
            "w1kv": np.ascontiguousarray(np.concatenate([W1[:, ksl], W1[:, vsl]], axis=1)),
            "b1q": np.ascontiguousarray(b1[qsl]),
            "b1kv": np.ascontiguousarray(np.concatenate([b1[ksl], b1[vsl]])),
            "w2": np.ascontiguousarray(W2[ksl, :].astype(BF)),
            "mask0": mask0,
            "mask1": mask1,
            "zer": zer,
        })
    return in_maps


def kernel(x, W1, b1, W2, b2):
    nc = _get_nc()
    in_maps = make_core_inputs(x, W1, b1, W2, b2)
    kwargs = {}
    if TRACE:
        kwargs = {"trace": True, "tmpdir": TRACE_DIR}
    res = run_bass_kernel_spmd(nc, in_maps, list(range(8)), **kwargs)
    LAST_RESULTS[0] = res
    b2 = np.asarray(b2, dtype=np.float32)
    out = np.zeros((B, T, D), np.float32)
    for c in range(8):
        out[c // 4] += np.asarray(res.results[c]["out"]).astype(np.float32).T
    out += b2[None, None, :]
    return out



# revision 8
# speedup vs baseline: 1.2651x; 1.1259x over previous
"""Trainium2 Bass kernel for causal softmax-free multi-head attention (retention).

Reference computation (per batch b):
    kqv = x @ W1 + b1 ; k, q, v = split(kqv, 3)   [split order k, q, v]
    per head h (dh = 64):  attn = tril(q_h @ k_h^T) ; o_h = attn @ v_h
    out = concat_h(o_h) @ W2 + b2

Sharding: 8 cores = 2 batches x 4 head-groups (4 heads each). Each core
computes its batch's projections restricted to its heads' weight columns,
the attention for its 4 heads, and a partial output projection
(out_local @ W2[rows of its heads]). Host sums the 4 partials per batch.

Algorithm: chunked linear attention. tril(QK^T)V is computed per 256-token
block as  O = Q @ S + tril_block(Q K_blk^T) V_blk, with the running state
S = K^T V accumulated over previous blocks ([64,64] per head).

All matmul operands are bf16 (rel err ~6e-3, gate is 2e-2); PSUM stays f32.

v2 layout strategy:
  - Q^T, K^T, V^T all computed feature-major in ap=512 waves from
    single-DMA weight tiles (w1 packed [1024, 768] K|V|Q on host).
  - Token-major K, V (needed for the state update / attn.V contractions)
    come from hardware DMA transposes (XBAR) of K^T/V^T: [128,2048] ->
    [128,16,128] tiled, 2 triggers per tensor, zero PE cost.
  - Scores contract K=64 directly (bf16 allows K<128) with partition-offset
    operands - no zero-padded K^T copies.
  - No SBUF pool aliasing (everything fits in ~12MB) so W2/masks/zeros
    load up front and phase transitions don't stall on WAR deps.
"""

import numpy as np

import concourse.bacc as bacc
import concourse.mybir as mybir
import concourse.tile as tile
from concourse.bass_utils import run_bass_kernel_spmd

F32 = mybir.dt.float32
BF16 = mybir.dt.bfloat16
AF = mybir.ActivationFunctionType

B, T, D = 2, 2048, 1024
H, DH = 16, 64
HPC = 4           # heads per core
FH = HPC * DH     # 256 features per core per tensor
BLK = 256         # state-update block (2 x 128-token chunks)
NBLK = T // BLK   # 8
ND = D // 128     # 8 contraction chunks
NQT = T // 512    # 4 wide token tiles

TRACE = False
TRACE_DIR = None
LAST_RESULTS = [None]


def _build():
    nc = bacc.Bacc("TRN2", target_bir_lowering=False, debug=False, num_devices=8)

    xT = nc.dram_tensor("xT", [D, T], BF16, kind="ExternalInput").ap()
    w1 = nc.dram_tensor("w1", [D, 3 * FH], BF16, kind="ExternalInput").ap()
    w2 = nc.dram_tensor("w2", [FH, D], BF16, kind="ExternalInput").ap()
    b1p = nc.dram_tensor("b1p", [128, 6], F32, kind="ExternalInput").ap()
    masks = nc.dram_tensor("masks", [128, 512], F32, kind="ExternalInput").ap()
    out = nc.dram_tensor("out", [D, T], BF16, kind="ExternalOutput").ap()

    with tile.TileContext(nc) as tc:
        with (
            tc.tile_pool(name="persist", bufs=1) as pp,
            tc.tile_pool(name="work", bufs=4) as wp,
            tc.tile_pool(name="psA", bufs=4, space="PSUM") as psA,
            tc.tile_pool(name="psO", bufs=2, space="PSUM") as psO,
            tc.tile_pool(name="psU", bufs=2, space="PSUM") as psU,
        ):
            # ---- persistent SBUF tiles -------------------------------------
            xt = [pp.tile([128, T], BF16, name=f"xt{i}", tag=f"xt{i}") for i in range(ND)]
            w1_sb = pp.tile([128, ND, 3 * FH], BF16, name="w1_sb", tag="w1_sb")
            w2_sb = pp.tile([128, 2, D], BF16, name="w2_sb", tag="w2_sb")
            b1_sb = pp.tile([128, 6], F32, name="b1_sb", tag="b1_sb")
            mk_sb = pp.tile([128, 512], F32, name="mk_sb", tag="mk_sb")
            kT = [pp.tile([128, T], BF16, name=f"kT{g}", tag=f"kT{g}") for g in range(2)]
            qT = [pp.tile([128, T], BF16, name=f"qT{g}", tag=f"qT{g}") for g in range(2)]
            vT = [pp.tile([128, T], BF16, name=f"vT{g}", tag=f"vT{g}") for g in range(2)]
            ktok = [pp.tile([128, 16, 128], BF16, name=f"ktok{g}", tag=f"ktok{g}") for g in range(2)]
            vtok = [pp.tile([128, 16, 128], BF16, name=f"vtok{g}", tag=f"vtok{g}") for g in range(2)]
            oT = [pp.tile([128, T], BF16, name=f"oT{g}", tag=f"oT{g}") for g in range(2)]
            spad = [pp.tile([128, 128], BF16, name=f"spad{h}", tag=f"spad{h}") for h in range(4)]
            a1ring = [pp.tile([128, 2 * BLK], BF16, name=f"a1r{i}", tag=f"a1r{i}")
                      for i in range(4)]

            # ---- input DMAs ------------------------------------------------
            # gpsimd queue: all the whole-tensor loads (single triggers).
            nc.gpsimd.dma_start(
                out=w1_sb[:], in_=w1.rearrange("(c p) f -> p c f", p=128))
            nc.gpsimd.dma_start(out=mk_sb[:], in_=masks)
            nc.gpsimd.dma_start(out=b1_sb[:], in_=b1p)
            nc.gpsimd.dma_start(
                out=w2_sb[:], in_=w2.rearrange("(c p) f -> p c f", p=128))
            # sync queue: x^T chunk halves, first halves first so wave A can
            # start as soon as w1 + xt[0] land.
            HT = T // 2
            for i in range(ND):
                nc.sync.dma_start(out=xt[i][:, 0:HT], in_=xT[128 * i:128 * (i + 1), 0:HT])
            for i in range(ND):
                nc.sync.dma_start(out=xt[i][:, HT:T], in_=xT[128 * i:128 * (i + 1), HT:T])

            # zero-fills via memset (no DMA traffic): state tiles and the
            # always-zero left halves of the a1 ring slots.
            for h in range(4):
                nc.vector.memset(spad[h][:], 0)
            for i in range(4):
                for par in range(2):
                    nc.gpsimd.memset(a1ring[i][:, par * BLK: par * BLK + 128], 0)

            # ---- phase B: projection waves ---------------------------------
            # f-tile order in w1 packing: k0 k1 v0 v1 q0 q1
            _pools = [(psA, "pa"), (psU, "pu"), (psO, "po"),
                      (psA, "pa"), (psU, "pu"), (psO, "po"),
                      (psA, "pa"), (psA, "pa")]

            def run_wave(groups):
                # groups: list of (ft, qt, copyback_dst_tile_or_None)
                tiles = []
                for gi, _ in enumerate(groups):
                    pool, tag = _pools[gi]
                    tiles.append(pool.tile([128, 512], F32, name=f"pw{gi}", tag=tag))
                for d in range(ND):
                    for gi, (ft, qt, _) in enumerate(groups):
                        nc.tensor.matmul(
                            tiles[gi][:],
                            w1_sb[:, d, ft * 128:(ft + 1) * 128],
                            xt[d][:, qt * 512:(qt + 1) * 512],
                            start=(d == 0), stop=(d == ND - 1))
                for gi, (ft, qt, dst) in enumerate(groups):
                    nc.scalar.activation(
                        dst[:, qt * 512:(qt + 1) * 512], tiles[gi][:],
                        AF.Identity, bias=b1_sb[:, ft:ft + 1])

            run_wave([(0, 0, kT[0]), (1, 0, kT[1]), (2, 0, vT[0]), (3, 0, vT[1]),
                      (0, 1, kT[0]), (1, 1, kT[1]), (2, 1, vT[0]), (3, 1, vT[1])])
            run_wave([(0, 2, kT[0]), (1, 2, kT[1]), (2, 2, vT[0]), (3, 2, vT[1]),
                      (0, 3, kT[0]), (1, 3, kT[1]), (2, 3, vT[0]), (3, 3, vT[1])])
            # token-major K/V via XBAR dma transpose. All four go on ONE
            # engine queue: concurrent XBAR transposes on two queues were
            # observed to corrupt the first chunks of the second transfer.
            nc.scalar.dma_start_transpose(out=ktok[0][:], in_=kT[0][:])
            nc.scalar.dma_start_transpose(out=ktok[1][:], in_=kT[1][:])
            nc.scalar.dma_start_transpose(out=vtok[0][:], in_=vT[0][:])
            nc.scalar.dma_start_transpose(out=vtok[1][:], in_=vT[1][:])
            run_wave([(4, 0, qT[0]), (5, 0, qT[1]), (4, 1, qT[0]), (5, 1, qT[1]),
                      (4, 2, qT[0]), (5, 2, qT[1]), (4, 3, qT[0]), (5, 3, qT[1])])

            # ---- phase C: chunked causal attention + interleaved D ---------
            # Two-stage software pipeline: block m's scores are emitted
            # before block m-1's O-accumulation chains, so the in-order PE
            # stream always has independent matmuls while the DVE applies
            # causal masks for the previous block.
            ablk = {}

            def scores_block(m):
                qsl = slice(m * BLK, (m + 1) * BLK)
                for pg in range(2):
                    a0 = wp.tile([128, 2 * BLK], BF16, name="a0", tag="a0", bufs=4)
                    a1 = a1ring[(2 * m + pg) % 4]
                    ablk[(m, pg)] = (a0, a1)
                    for par in range(2):
                        rows = slice(par * 64, (par + 1) * 64)
                        asl = slice(par * BLK, (par + 1) * BLK)
                        pA = psA.tile([128, 2 * BLK], F32, name="pA", tag="pa")
                        nc.tensor.matmul(
                            pA[:, 0:BLK],
                            kT[pg][rows, (2 * m) * 128:(2 * m + 1) * 128],
                            qT[pg][rows, qsl], start=True, stop=True)
                        nc.tensor.matmul(
                            pA[:, BLK:2 * BLK],
                            kT[pg][rows, (2 * m + 1) * 128:(2 * m + 2) * 128],
                            qT[pg][rows, qsl], start=True, stop=True,
                            skip_group_check=True)
                        nc.vector.tensor_tensor(
                            a0[:, asl], pA[:, 0:BLK], mk_sb[:, 0:BLK],
                            mybir.AluOpType.mult)
                        # only the tril half: the left 128 cols stay zero
                        nc.vector.tensor_tensor(
                            a1[:, par * BLK + 128:(par + 1) * BLK],
                            pA[:, BLK + 128:2 * BLK], mk_sb[:, BLK + 128:2 * BLK],
                            mybir.AluOpType.mult)

            def chains_block(m):
                qsl = slice(m * BLK, (m + 1) * BLK)
                for pg in range(2):
                    a0, a1 = ablk.pop((m, pg))
                    pO = psO.tile([128, 2 * BLK], F32, name="pO", tag="po")
                    nc.tensor.matmul(
                        pO[:], vtok[pg][:, 2 * m, :], a0[:],
                        start=True, stop=False)
                    nc.tensor.matmul(
                        pO[:], vtok[pg][:, 2 * m + 1, :], a1[:],
                        start=False, stop=(m == 0))
                    if m > 0:
                        nc.tensor.matmul(
                            pO[:, 0:BLK], spad[2 * pg][:], qT[pg][:, qsl],
                            start=False, stop=False)
                        nc.tensor.matmul(
                            pO[:, BLK:2 * BLK], spad[2 * pg + 1][:], qT[pg][:, qsl],
                            start=False, stop=True)
                    for par in range(2):
                        hr = slice(par * 64, (par + 1) * 64)
                        nc.scalar.activation(
                            oT[pg][hr, qsl],
                            pO[hr, par * BLK:par * BLK + BLK], AF.Identity)

                for pg in range(2):
                    pU = psU.tile([128, 128], F32, name="pU", tag="pu")
                    nc.tensor.matmul(
                        pU[:], ktok[pg][:, 2 * m, :], vtok[pg][:, 2 * m, :],
                        start=True, stop=False)
                    nc.tensor.matmul(
                        pU[:], ktok[pg][:, 2 * m + 1, :], vtok[pg][:, 2 * m + 1, :],
                        start=False, stop=True)
                    for par in range(2):
                        h = 2 * pg + par
                        hr = slice(par * 64, (par + 1) * 64)
                        nc.vector.tensor_tensor(
                            spad[h][hr, hr], pU[hr, hr],
                            spad[h][hr, hr], mybir.AluOpType.add)

            def proj_tile(qt, half):
                dcr = range(0, ND // 2) if half == 0 else range(ND // 2, ND)
                for dc in dcr:
                    pf = [psA.tile([128, 512], F32, name="pf", tag="pa"),
                          psU.tile([128, 512], F32, name="pf2", tag="pu"),
                          psO.tile([128, 512], F32, name="pf3", tag="po")][dc % 3]
                    for g2 in range(2):
                        nc.tensor.matmul(
                            pf[:],
                            w2_sb[:, g2, dc * 128:(dc + 1) * 128],
                            oT[g2][:, qt * 512:(qt + 1) * 512],
                            start=(g2 == 0), stop=(g2 == 1))
                    fs = wp.tile([128, 512], BF16, name="fs", tag="fs", bufs=3)
                    if dc % 2 == 0:
                        nc.vector.tensor_copy(fs[:], pf[:])
                    else:
                        nc.scalar.activation(fs[:], pf[:], AF.Identity)
                    dma_eng = nc.gpsimd if dc % 2 == 0 else nc.sync
                    dma_eng.dma_start(
                        out=out[dc * 128:(dc + 1) * 128, qt * 512:(qt + 1) * 512],
                        in_=fs[:])

            # proj_tile(qt) is emitted one-plus blocks after the chains that
            # produce its oT inputs, so the PE never waits on the Scalar
            # engine's oT copybacks.
            dplan = {3: [(0, 0)], 4: [(0, 1)], 5: [(1, 0)], 6: [(1, 1)],
                     7: [(2, 0), (2, 1)]}
            scores_block(0)
            for m in range(1, NBLK):
                scores_block(m)
                chains_block(m - 1)
                for pt in dplan.get(m, []):
                    proj_tile(*pt)
            chains_block(NBLK - 1)
            proj_tile(3, 0)
            proj_tile(3, 1)

    nc.compile()
    return nc


_NC = None


def _get_nc():
    global _NC
    if _NC is None:
        _NC = _build()
    return _NC


def make_core_inputs(x, W1, b1, W2, b2):
    """Shard full inputs into the 8 per-core input dicts."""
    import ml_dtypes
    BF = ml_dtypes.bfloat16
    x = np.asarray(x, dtype=np.float32)
    W1 = np.asarray(W1, dtype=np.float32).astype(BF)
    b1 = np.asarray(b1, dtype=np.float32)
    W2 = np.asarray(W2, dtype=np.float32)

    p = np.arange(128)[:, None]
    f = np.arange(BLK)[None, :]
    mask0 = (f >= p).astype(np.float32)
    mask1 = (f >= p + 128).astype(np.float32)
    masks = np.concatenate([mask0, mask1], axis=1)

    in_maps = []
    for c in range(8):
        b = c // 4
        g = c % 4
        ksl = slice(g * FH, (g + 1) * FH)
        qsl = slice(D + g * FH, D + (g + 1) * FH)
        vsl = slice(2 * D + g * FH, 2 * D + (g + 1) * FH)
        # w1 packed K | V | Q along features (matches f-tile order k0 k1 v0 v1 q0 q1)
        w1p = np.concatenate([W1[:, ksl], W1[:, vsl], W1[:, qsl]], axis=1)
        b1loc = np.concatenate([b1[ksl], b1[vsl], b1[qsl]])
        b1p = np.ascontiguousarray(b1loc.reshape(6, 128).T.astype(np.float32))
        in_maps.append({
            "xT": np.ascontiguousarray(x[b].T.astype(BF)),
            "w1": np.ascontiguousarray(w1p),
            "w2": np.ascontiguousarray(W2[ksl, :].astype(BF)),
            "b1p": b1p,
            "masks": masks,
        })
    return in_maps


def kernel(x, W1, b1, W2, b2):
    nc = _get_nc()
    in_maps = make_core_inputs(x, W1, b1, W2, b2)
    kwargs = {}
    if TRACE:
        kwargs = {"trace": True, "tmpdir": TRACE_DIR}
    res = run_bass_kernel_spmd(nc, in_maps, list(range(8)), **kwargs)
    LAST_RESULTS[0] = res
    b2 = np.asarray(b2, dtype=np.float32)
    out = np.zeros((B, T, D), np.float32)
    for c in range(8):
        out[c // 4] += np.asarray(res.results[c]["out"]).astype(np.float32).T
    out += b2[None, None, :]
    return out


# revision 15
# speedup vs baseline: 1.4486x; 1.1451x over previous
"""Trainium2 Bass kernel for causal softmax-free multi-head attention (retention).

Reference computation (per batch b):
    kqv = x @ W1 + b1 ; k, q, v = split(kqv, 3)   [split order k, q, v]
    per head h (dh = 64):  attn = tril(q_h @ k_h^T) ; o_h = attn @ v_h
    out = concat_h(o_h) @ W2 + b2

Sharding: 8 cores = 2 batches x 4 head-groups (4 heads each). Each core
computes its batch's projections restricted to its heads' weight columns,
the attention for its 4 heads, and a partial output projection
(out_local @ W2[rows of its heads]). Host sums the 4 partials per batch.

Algorithm: chunked linear attention. tril(QK^T)V is computed per 256-token
block as  O = Q @ S + tril_block(Q K_blk^T) V_blk, with the running state
S = K^T V accumulated over previous blocks ([64,64] per head).

All matmul operands are bf16 (rel err ~6e-3, gate is 2e-2); PSUM stays f32.

v2 layout strategy:
  - Q^T, K^T, V^T all computed feature-major in ap=512 waves from
    single-DMA weight tiles (w1 packed [1024, 768] K|V|Q on host).
  - Token-major K, V (needed for the state update / attn.V contractions)
    come from hardware DMA transposes (XBAR) of K^T/V^T: [128,2048] ->
    [128,16,128] tiled, 2 triggers per tensor, zero PE cost.
  - Scores contract K=64 directly (bf16 allows K<128) with partition-offset
    operands - no zero-padded K^T copies.
  - No SBUF pool aliasing (everything fits in ~12MB) so W2/masks/zeros
    load up front and phase transitions don't stall on WAR deps.
"""

import numpy as np

import concourse.bacc as bacc
import concourse.mybir as mybir
import concourse.tile as tile
from concourse.bass_utils import run_bass_kernel_spmd

F32 = mybir.dt.float32
BF16 = mybir.dt.bfloat16
AF = mybir.ActivationFunctionType

B, T, D = 2, 2048, 1024
H, DH = 16, 64
HPC = 4           # heads per core
FH = HPC * DH     # 256 features per core per tensor
BLK = 256         # state-update block (2 x 128-token chunks)
NBLK = T // BLK   # 8
ND = D // 128     # 8 contraction chunks
NQT = T // 512    # 4 wide token tiles

TRACE = False
TRACE_DIR = None
LAST_RESULTS = [None]


def _build():
    nc = bacc.Bacc("TRN2", target_bir_lowering=False, debug=False, num_devices=8)

    xT = nc.dram_tensor("xT", [D, T], BF16, kind="ExternalInput").ap()
    w1 = nc.dram_tensor("w1", [D, 3 * FH], BF16, kind="ExternalInput").ap()
    w2 = nc.dram_tensor("w2", [FH, D], BF16, kind="ExternalInput").ap()
    b1p = nc.dram_tensor("b1p", [128, 6], F32, kind="ExternalInput").ap()
    masks = nc.dram_tensor("masks", [128, 512], F32, kind="ExternalInput").ap()
    # output chunk-major: out[p, dc, t] = full_out[dc*128 + p, t]; host reassembles
    out = nc.dram_tensor("out", [128, ND * T], BF16, kind="ExternalOutput").ap()
    out3 = out.rearrange("p (c t) -> p c t", c=ND)

    with tile.TileContext(nc) as tc:
        with (
            tc.tile_pool(name="persist", bufs=1) as pp,
            tc.tile_pool(name="work", bufs=4) as wp,
            tc.tile_pool(name="psA", bufs=4, space="PSUM") as psA,
            tc.tile_pool(name="psO", bufs=2, space="PSUM") as psO,
            tc.tile_pool(name="psU", bufs=2, space="PSUM") as psU,
        ):
            # ---- persistent SBUF tiles -------------------------------------
            xt = [pp.tile([128, T], BF16, name=f"xt{i}", tag=f"xt{i}") for i in range(ND)]
            w1d = [pp.tile([128, 3 * FH], BF16, name=f"w1d{i}", tag=f"w1d{i}")
                   for i in range(ND)]
            w2_sb = pp.tile([128, 2, D], BF16, name="w2_sb", tag="w2_sb")
            b1_sb = pp.tile([128, 6], F32, name="b1_sb", tag="b1_sb")
            mk_sb = pp.tile([128, 512], F32, name="mk_sb", tag="mk_sb")
            kT = [pp.tile([128, T], BF16, name=f"kT{g}", tag=f"kT{g}") for g in range(2)]
            qT = [pp.tile([128, T], BF16, name=f"qT{g}", tag=f"qT{g}") for g in range(2)]
            vT = [pp.tile([128, T], BF16, name=f"vT{g}", tag=f"vT{g}") for g in range(2)]
            ktok = [pp.tile([128, 16, 128], BF16, name=f"ktok{g}", tag=f"ktok{g}") for g in range(2)]
            vtok = [pp.tile([128, 16, 128], BF16, name=f"vtok{g}", tag=f"vtok{g}") for g in range(2)]
            oT = [pp.tile([128, T], BF16, name=f"oT{g}", tag=f"oT{g}") for g in range(2)]
            spad = [pp.tile([128, 128], BF16, name=f"spad{h}", tag=f"spad{h}") for h in range(4)]
            a1ring = [pp.tile([128, 2 * BLK], BF16, name=f"a1r{i}", tag=f"a1r{i}")
                      for i in range(4)]

            # ---- input DMAs ------------------------------------------------
            # gpsimd queue: per-d-chunk w1 tiles first (wave A starts after
            # just w1d[0] + xt[0]'s first half), then the small tensors.
            for i in range(ND):
                nc.gpsimd.dma_start(
                    out=w1d[i][:], in_=w1[128 * i:128 * (i + 1), :])
            nc.gpsimd.dma_start(out=mk_sb[:], in_=masks)
            nc.gpsimd.dma_start(out=b1_sb[:], in_=b1p)
            nc.gpsimd.dma_start(
                out=w2_sb[:], in_=w2.rearrange("(c p) f -> p c f", p=128))
            # sync queue: x^T chunk halves, first halves first so wave A can
            # start as soon as w1d[0] + xt[0] land.
            HT = T // 2
            for i in range(ND):
                nc.sync.dma_start(out=xt[i][:, 0:HT], in_=xT[128 * i:128 * (i + 1), 0:HT])
            for i in range(ND):
                nc.sync.dma_start(out=xt[i][:, HT:T], in_=xT[128 * i:128 * (i + 1), HT:T])

            # zero-fills via memset (no DMA traffic): state tiles and the
            # always-zero left halves of the a1 ring slots.
            for h in range(4):
                nc.vector.memset(spad[h][:], 0)
            for i in range(4):
                for par in range(2):
                    nc.gpsimd.memset(a1ring[i][:, par * BLK: par * BLK + 128], 0)

            # ---- phase B: projection waves ---------------------------------
            # f-tile order in w1 packing: k0 k1 v0 v1 q0 q1
            _pools = [(psA, "pa"), (psU, "pu"), (psO, "po"),
                      (psA, "pa"), (psU, "pu"), (psO, "po"),
                      (psA, "pa"), (psA, "pa")]

            def run_wave(groups):
                # groups: list of (ft, qt, copyback_dst_tile_or_None)
                tiles = []
                for gi, _ in enumerate(groups):
                    pool, tag = _pools[gi]
                    tiles.append(pool.tile([128, 512], F32, name=f"pw{gi}", tag=tag))
                for d in range(ND):
                    for gi, (ft, qt, _) in enumerate(groups):
                        nc.tensor.matmul(
                            tiles[gi][:],
                            w1d[d][:, ft * 128:(ft + 1) * 128],
                            xt[d][:, qt * 512:(qt + 1) * 512],
                            start=(d == 0), stop=(d == ND - 1))
                for gi, (ft, qt, dst) in enumerate(groups):
                    nc.scalar.activation(
                        dst[:, qt * 512:(qt + 1) * 512], tiles[gi][:],
                        AF.Identity, bias=b1_sb[:, ft:ft + 1])

            run_wave([(0, 0, kT[0]), (1, 0, kT[1]), (2, 0, vT[0]), (3, 0, vT[1]),
                      (0, 1, kT[0]), (1, 1, kT[1]), (2, 1, vT[0]), (3, 1, vT[1])])
            run_wave([(0, 2, kT[0]), (1, 2, kT[1]), (2, 2, vT[0]), (3, 2, vT[1]),
                      (0, 3, kT[0]), (1, 3, kT[1]), (2, 3, vT[0]), (3, 3, vT[1])])
            # token-major K/V via XBAR dma transpose. All four go on ONE
            # engine queue: concurrent XBAR transposes on two queues were
            # observed to corrupt the first chunks of the second transfer.
            # They live on sync (idle by now) so the scalar engine's ACT
            # copyback stream is not delayed.
            nc.sync.dma_start_transpose(out=vtok[0][:], in_=vT[0][:])
            nc.sync.dma_start_transpose(out=vtok[1][:], in_=vT[1][:])
            nc.sync.dma_start_transpose(out=ktok[0][:], in_=kT[0][:])
            nc.sync.dma_start_transpose(out=ktok[1][:], in_=kT[1][:])
            run_wave([(4, 0, qT[0]), (5, 0, qT[1]), (4, 1, qT[0]), (5, 1, qT[1]),
                      (4, 2, qT[0]), (5, 2, qT[1]), (4, 3, qT[0]), (5, 3, qT[1])])

            # ---- phase C: chunked causal attention + interleaved D ---------
            # Two-stage software pipeline: block m's scores are emitted
            # before block m-1's O-accumulation chains, so the in-order PE
            # stream always has independent matmuls while the DVE applies
            # causal masks for the previous block.
            ablk = {}

            def scores_block(m):
                qsl = slice(m * BLK, (m + 1) * BLK)
                for pg in range(2):
                    a0 = wp.tile([128, 2 * BLK], BF16, name="a0", tag="a0", bufs=4)
                    a1 = a1ring[(2 * m + pg) % 4]
                    ablk[(m, pg)] = (a0, a1)
                    for par in range(2):
                        rows = slice(par * 64, (par + 1) * 64)
                        asl = slice(par * BLK, (par + 1) * BLK)
                        pA = psA.tile([128, 2 * BLK], F32, name="pA", tag="pa")
                        nc.tensor.matmul(
                            pA[:, 0:BLK],
                            kT[pg][rows, (2 * m) * 128:(2 * m + 1) * 128],
                            qT[pg][rows, qsl], start=True, stop=True)
                        nc.tensor.matmul(
                            pA[:, BLK:2 * BLK],
                            kT[pg][rows, (2 * m + 1) * 128:(2 * m + 2) * 128],
                            qT[pg][rows, qsl], start=True, stop=True,
                            skip_group_check=True)
                        nc.vector.tensor_tensor(
                            a0[:, asl], pA[:, 0:BLK], mk_sb[:, 0:BLK],
                            mybir.AluOpType.mult)
                        # only the tril half: the left 128 cols stay zero
                        nc.vector.tensor_tensor(
                            a1[:, par * BLK + 128:(par + 1) * BLK],
                            pA[:, BLK + 128:2 * BLK], mk_sb[:, BLK + 128:2 * BLK],
                            mybir.AluOpType.mult)

            def chains_block(m):
                qsl = slice(m * BLK, (m + 1) * BLK)
                for pg in range(2):
                    a0, a1 = ablk.pop((m, pg))
                    pO = psO.tile([128, 2 * BLK], F32, name="pO", tag="po")
                    nc.tensor.matmul(
                        pO[:], vtok[pg][:, 2 * m, :], a0[:],
                        start=True, stop=False)
                    nc.tensor.matmul(
                        pO[:], vtok[pg][:, 2 * m + 1, :], a1[:],
                        start=False, stop=(m == 0))
                    if m > 0:
                        nc.tensor.matmul(
                            pO[:, 0:BLK], spad[2 * pg][:], qT[pg][:, qsl],
                            start=False, stop=False)
                        nc.tensor.matmul(
                            pO[:, BLK:2 * BLK], spad[2 * pg + 1][:], qT[pg][:, qsl],
                            start=False, stop=True)
                    for par in range(2):
                        hr = slice(par * 64, (par + 1) * 64)
                        nc.scalar.activation(
                            oT[pg][hr, qsl],
                            pO[hr, par * BLK:par * BLK + BLK], AF.Identity)

                for pg in range(2):
                    pU = psU.tile([128, 128], F32, name="pU", tag="pu")
                    nc.tensor.matmul(
                        pU[:], ktok[pg][:, 2 * m, :], vtok[pg][:, 2 * m, :],
                        start=True, stop=False)
                    nc.tensor.matmul(
                        pU[:], ktok[pg][:, 2 * m + 1, :], vtok[pg][:, 2 * m + 1, :],
                        start=False, stop=True)
                    for par in range(2):
                        h = 2 * pg + par
                        hr = slice(par * 64, (par + 1) * 64)
                        nc.vector.tensor_tensor(
                            spad[h][hr, hr], pU[hr, hr],
                            spad[h][hr, hr], mybir.AluOpType.add)

            # output staging: all 8 dout-chunks of a qt window accumulate in
            # one [128, ND, 512] tile, shipped with a single DMA per qt.
            fso_by_qt = {}

            def proj_tile(qt, half):
                if half == 0:
                    fso_by_qt[qt] = wp.tile([128, ND, 512], BF16, name="fso",
                                            tag="fso", bufs=2)
                fso = fso_by_qt[qt]
                dcr = range(0, ND // 2) if half == 0 else range(ND // 2, ND)
                for dc in dcr:
                    pf = [psA.tile([128, 512], F32, name="pf", tag="pa"),
                          psU.tile([128, 512], F32, name="pf2", tag="pu"),
                          psO.tile([128, 512], F32, name="pf3", tag="po")][dc % 3]
                    for g2 in range(2):
                        nc.tensor.matmul(
                            pf[:],
                            w2_sb[:, g2, dc * 128:(dc + 1) * 128],
                            oT[g2][:, qt * 512:(qt + 1) * 512],
                            start=(g2 == 0), stop=(g2 == 1))
                    if dc % 2 == 0:
                        nc.vector.tensor_copy(fso[:, dc, :], pf[:])
                    else:
                        nc.scalar.activation(fso[:, dc, :], pf[:], AF.Identity)
                if half == 1:
                    del fso_by_qt[qt]
                    dma_eng = nc.gpsimd if qt % 2 == 0 else nc.sync
                    dma_eng.dma_start(
                        out=out3[:, :, qt * 512:(qt + 1) * 512], in_=fso[:])

            # proj_tile(qt) is emitted one-plus blocks after the chains that
            # produce its oT inputs, so the PE never waits on the Scalar
            # engine's oT copybacks.
            dplan = {3: [(0, 0)], 4: [(0, 1)], 5: [(1, 0)], 6: [(1, 1)],
                     7: [(2, 0), (2, 1)]}
            scores_block(0)
            for m in range(1, NBLK):
                scores_block(m)
                chains_block(m - 1)
                for pt in dplan.get(m, []):
                    proj_tile(*pt)
            chains_block(NBLK - 1)
            proj_tile(3, 0)
            proj_tile(3, 1)

    nc.compile()
    return nc


_NC = None


def _get_nc():
    global _NC
    if _NC is None:
        _NC = _build()
    return _NC


def make_core_inputs(x, W1, b1, W2, b2):
    """Shard full inputs into the 8 per-core input dicts."""
    import ml_dtypes
    BF = ml_dtypes.bfloat16
    x = np.asarray(x, dtype=np.float32)
    W1 = np.asarray(W1, dtype=np.float32).astype(BF)
    b1 = np.asarray(b1, dtype=np.float32)
    W2 = np.asarray(W2, dtype=np.float32)

    p = np.arange(128)[:, None]
    f = np.arange(BLK)[None, :]
    mask0 = (f >= p).astype(np.float32)
    mask1 = (f >= p + 128).astype(np.float32)
    masks = np.concatenate([mask0, mask1], axis=1)

    in_maps = []
    for c in range(8):
        b = c // 4
        g = c % 4
        ksl = slice(g * FH, (g + 1) * FH)
        qsl = slice(D + g * FH, D + (g + 1) * FH)
        vsl = slice(2 * D + g * FH, 2 * D + (g + 1) * FH)
        # w1 packed K | V | Q along features (matches f-tile order k0 k1 v0 v1 q0 q1)
        w1p = np.concatenate([W1[:, ksl], W1[:, vsl], W1[:, qsl]], axis=1)
        b1loc = np.concatenate([b1[ksl], b1[vsl], b1[qsl]])
        b1p = np.ascontiguousarray(b1loc.reshape(6, 128).T.astype(np.float32))
        in_maps.append({
            "xT": np.ascontiguousarray(x[b].T.astype(BF)),
            "w1": np.ascontiguousarray(w1p),
            "w2": np.ascontiguousarray(W2[ksl, :].astype(BF)),
            "b1p": b1p,
            "masks": masks,
        })
    return in_maps


def kernel(x, W1, b1, W2, b2):
    nc = _get_nc()
    in_maps = make_core_inputs(x, W1, b1, W2, b2)
    kwargs = {}
    if TRACE:
        kwargs = {"trace": True, "tmpdir": TRACE_DIR}
    res = run_bass_kernel_spmd(nc, in_maps, list(range(8)), **kwargs)
    LAST_RESULTS[0] = res
    b2 = np.asarray(b2, dtype=np.float32)
    out = np.zeros((B, T, D), np.float32)
    for c in range(8):
        oc = np.asarray(res.results[c]["out"]).astype(np.float32)
        oc = oc.reshape(128, ND, T).transpose(1, 0, 2).reshape(D, T)
        out[c // 4] += oc.T
    out += b2[None, None, :]
    return out


# revision 31
# speedup vs baseline: 1.4695x; 1.0144x over previous
"""Trainium2 Bass kernel for causal softmax-free multi-head attention (retention).

Reference computation (per batch b):
    kqv = x @ W1 + b1 ; k, q, v = split(kqv, 3)   [split order k, q, v]
    per head h (dh = 64):  attn = tril(q_h @ k_h^T) ; o_h = attn @ v_h
    out = concat_h(o_h) @ W2 + b2

Sharding: 8 cores = 2 batches x 4 head-groups (4 heads each). Each core
computes its batch's projections restricted to its heads' weight columns,
the attention for its 4 heads, and a partial output projection
(out_local @ W2[rows of its heads]). Host sums the 4 partials per batch.

Algorithm: chunked linear attention. tril(QK^T)V is computed per 256-token
block as  O = Q @ S + tril_block(Q K_blk^T) V_blk, with the running state
S = K^T V accumulated over previous blocks ([64,64] per head).

All matmul operands are bf16 (rel err ~6e-3, gate is 2e-2); PSUM stays f32.

v2 layout strategy:
  - Q^T, K^T, V^T all computed feature-major in ap=512 waves from
    single-DMA weight tiles (w1 packed [1024, 768] K|V|Q on host).
  - Token-major K, V (needed for the state update / attn.V contractions)
    come from hardware DMA transposes (XBAR) of K^T/V^T: [128,2048] ->
    [128,16,128] tiled, 2 triggers per tensor, zero PE cost.
  - Scores contract K=64 directly (bf16 allows K<128) with partition-offset
    operands - no zero-padded K^T copies.
  - No SBUF pool aliasing (everything fits in ~12MB) so W2/masks/zeros
    load up front and phase transitions don't stall on WAR deps.
"""

import numpy as np

import concourse.bacc as bacc
import concourse.mybir as mybir
import concourse.tile as tile
from concourse.bass_utils import run_bass_kernel_spmd

F32 = mybir.dt.float32
BF16 = mybir.dt.bfloat16
AF = mybir.ActivationFunctionType

B, T, D = 2, 2048, 1024
H, DH = 16, 64
HPC = 4           # heads per core
FH = HPC * DH     # 256 features per core per tensor
BLK = 256         # state-update block (2 x 128-token chunks)
NBLK = T // BLK   # 8
ND = D // 128     # 8 contraction chunks
NQT = T // 512    # 4 wide token tiles

TRACE = False
TRACE_DIR = None
LAST_RESULTS = [None]


def _build():
    nc = bacc.Bacc("TRN2", target_bir_lowering=False, debug=False, num_devices=8)

    xT = nc.dram_tensor("xT", [D, T], BF16, kind="ExternalInput").ap()
    w1 = nc.dram_tensor("w1", [D, 3 * FH], BF16, kind="ExternalInput").ap()
    w2 = nc.dram_tensor("w2", [FH, D], BF16, kind="ExternalInput").ap()
    b1p = nc.dram_tensor("b1p", [128, 6], F32, kind="ExternalInput").ap()
    masks = nc.dram_tensor("masks", [128, 512], F32, kind="ExternalInput").ap()
    # output chunk-major: out[p, dc, t] = full_out[dc*128 + p, t]; host reassembles
    out = nc.dram_tensor("out", [128, ND * T], BF16, kind="ExternalOutput").ap()
    out3 = out.rearrange("p (c t) -> p c t", c=ND)

    with tile.TileContext(nc) as tc:
        with (
            tc.tile_pool(name="persist", bufs=1) as pp,
            tc.tile_pool(name="work", bufs=4) as wp,
            tc.tile_pool(name="psA", bufs=4, space="PSUM") as psA,
            tc.tile_pool(name="psO", bufs=2, space="PSUM") as psO,
            tc.tile_pool(name="psU", bufs=2, space="PSUM") as psU,
        ):
            # ---- persistent SBUF tiles -------------------------------------
            xt = [pp.tile([128, T], BF16, name=f"xt{i}", tag=f"xt{i}") for i in range(ND)]
            w1d = [pp.tile([128, 3 * FH], BF16, name=f"w1d{i}", tag=f"w1d{i}")
                   for i in range(ND)]
            w2_sb = pp.tile([128, 2, D], BF16, name="w2_sb", tag="w2_sb")
            b1_sb = pp.tile([128, 6], F32, name="b1_sb", tag="b1_sb")
            mk_sb = pp.tile([128, 512], F32, name="mk_sb", tag="mk_sb")
            kT = [pp.tile([128, T], BF16, name=f"kT{g}", tag=f"kT{g}") for g in range(2)]
            qT = [pp.tile([128, T], BF16, name=f"qT{g}", tag=f"qT{g}") for g in range(2)]
            vT = [pp.tile([128, T], BF16, name=f"vT{g}", tag=f"vT{g}") for g in range(2)]
            ktok = [pp.tile([128, 16, 128], BF16, name=f"ktok{g}", tag=f"ktok{g}") for g in range(2)]
            vtok = [pp.tile([128, 16, 128], BF16, name=f"vtok{g}", tag=f"vtok{g}") for g in range(2)]
            oT = [pp.tile([128, T], BF16, name=f"oT{g}", tag=f"oT{g}") for g in range(2)]
            spad = [pp.tile([128, 128], BF16, name=f"spad{h}", tag=f"spad{h}") for h in range(4)]

            # ---- input DMAs ------------------------------------------------
            # gpsimd queue: per-d-chunk w1 tiles first (wave A starts after
            # just w1d[0] + xt[0]'s first half), then the small tensors.
            for i in range(ND):
                nc.gpsimd.dma_start(
                    out=w1d[i][:], in_=w1[128 * i:128 * (i + 1), :])
            nc.gpsimd.dma_start(out=mk_sb[:], in_=masks)
            nc.gpsimd.dma_start(out=b1_sb[:], in_=b1p)
            nc.gpsimd.dma_start(
                out=w2_sb[:], in_=w2.rearrange("(c p) f -> p c f", p=128))
            # sync queue: x^T chunk halves, first halves first so wave A can
            # start as soon as w1d[0] + xt[0] land.
            HT = T // 2
            for i in range(ND):
                nc.sync.dma_start(out=xt[i][:, 0:HT], in_=xT[128 * i:128 * (i + 1), 0:HT])
            for i in range(ND):
                nc.sync.dma_start(out=xt[i][:, HT:T], in_=xT[128 * i:128 * (i + 1), HT:T])

            # zero-fills via memset (no DMA traffic): state tiles
            for h in range(4):
                nc.vector.memset(spad[h][:], 0)

            # PE warm-up: ~3.5us of dummy matmuls during the initial DMA wait
            # so the tensor engine is at full clock when wave A starts.
            wu_w = pp.tile([128, 128], BF16, name="wu_w", tag="wu_w")
            wu_x = pp.tile([128, 512], BF16, name="wu_x", tag="wu_x")
            nc.vector.memset(wu_w[:], 0)
            nc.vector.memset(wu_x[:], 0)
            wu_p = psO.tile([128, 512], F32, name="wu_p", tag="po")
            for _ in range(16):
                nc.tensor.matmul(wu_p[:], wu_w[:], wu_x[:],
                                 start=True, stop=True, skip_group_check=True)

            # ---- phase B: projection waves ---------------------------------
            # f-tile order in w1 packing: k0 k1 v0 v1 q0 q1
            _pools = [(psA, "pa"), (psU, "pu"), (psO, "po"),
                      (psA, "pa"), (psU, "pu"), (psO, "po"),
                      (psA, "pa"), (psA, "pa")]

            def run_wave(groups):
                # groups: list of (ft, qt, copyback_dst_tile)
                tiles = []
                for gi, _ in enumerate(groups):
                    pool, tag = _pools[gi]
                    tiles.append(pool.tile([128, 512], F32, name=f"pw{gi}", tag=tag))
                for d in range(ND):
                    for gi, (ft, qt, dst) in enumerate(groups):
                        nc.tensor.matmul(
                            tiles[gi][:],
                            w1d[d][:, ft * 128:(ft + 1) * 128],
                            xt[d][:, qt * 512:(qt + 1) * 512],
                            start=(d == 0), stop=(d == ND - 1))
                        if d == ND - 1:
                            # copyback immediately after this group's last
                            # accumulation so the ACT stream starts early
                            nc.scalar.activation(
                                dst[:, qt * 512:(qt + 1) * 512], tiles[gi][:],
                                AF.Identity, bias=b1_sb[:, ft:ft + 1])

            run_wave([(0, 0, kT[0]), (1, 0, kT[1]), (2, 0, vT[0]), (3, 0, vT[1]),
                      (0, 1, kT[0]), (1, 1, kT[1]), (2, 1, vT[0]), (3, 1, vT[1])])
            run_wave([(0, 2, kT[0]), (1, 2, kT[1]), (2, 2, vT[0]), (3, 2, vT[1]),
                      (0, 3, kT[0]), (1, 3, kT[1]), (2, 3, vT[0]), (3, 3, vT[1])])
            # token-major K/V via XBAR dma transpose. All four go on ONE
            # engine queue: concurrent XBAR transposes on two queues were
            # observed to corrupt the first chunks of the second transfer.
            # They live on sync (idle by now) so the scalar engine's ACT
            # copyback stream is not delayed.
            nc.sync.dma_start_transpose(out=vtok[0][:], in_=vT[0][:])
            nc.sync.dma_start_transpose(out=vtok[1][:], in_=vT[1][:])
            nc.sync.dma_start_transpose(out=ktok[0][:], in_=kT[0][:])
            nc.sync.dma_start_transpose(out=ktok[1][:], in_=kT[1][:])
            run_wave([(4, 0, qT[0]), (5, 0, qT[1]), (4, 1, qT[0]), (5, 1, qT[1]),
                      (4, 2, qT[0]), (5, 2, qT[1]), (4, 3, qT[0]), (5, 3, qT[1])])

            # ---- phase C: chunked causal attention + interleaved D ---------
            # Two-stage software pipeline: block m's scores are emitted
            # before block m-1's O-accumulation chains, so the in-order PE
            # stream always has independent matmuls while the DVE applies
            # causal masks for the previous block.
            ablk = {}

            def scores_block(m):
                qsl = slice(m * BLK, (m + 1) * BLK)
                for pg in range(2):
                    a0 = wp.tile([128, 2 * BLK], BF16, name="a0", tag="a0", bufs=4)
                    # a1 packed [128, 256]: par's valid (below-diagonal) half
                    # of the chunk1 scores at cols par*128
                    a1 = wp.tile([128, BLK], BF16, name="a1", tag="a1", bufs=4)
                    ablk[(m, pg)] = (a0, a1)
                    for par in range(2):
                        rows = slice(par * 64, (par + 1) * 64)
                        asl = slice(par * BLK, (par + 1) * BLK)
                        pA = psA.tile([128, 2 * BLK], F32, name="pA", tag="pa")
                        nc.tensor.matmul(
                            pA[:, 0:BLK],
                            kT[pg][rows, (2 * m) * 128:(2 * m + 1) * 128],
                            qT[pg][rows, qsl], start=True, stop=True)
                        # chunk1 scores: only q-cols 128:256 of the window are
                        # below the diagonal; compute just those (ap=128).
                        nc.tensor.matmul(
                            pA[:, BLK + 128:2 * BLK],
                            kT[pg][rows, (2 * m + 1) * 128:(2 * m + 2) * 128],
                            qT[pg][rows, m * BLK + 128:(m + 1) * BLK],
                            start=True, stop=True, skip_group_check=True)
                        nc.vector.tensor_tensor(
                            a0[:, asl], pA[:, 0:BLK], mk_sb[:, 0:BLK],
                            mybir.AluOpType.mult)
                        nc.vector.tensor_tensor(
                            a1[:, par * 128:(par + 1) * 128],
                            pA[:, BLK + 128:2 * BLK], mk_sb[:, BLK + 128:2 * BLK],
                            mybir.AluOpType.mult)

            def chains_block(m):
                qsl = slice(m * BLK, (m + 1) * BLK)
                for pg in range(2):
                    a0, a1 = ablk.pop((m, pg))
                    pO = psO.tile([128, 2 * BLK], F32, name="pO", tag="po")
                    pO3 = pO.rearrange("p (c b) -> p c b", b=128)
                    nc.tensor.matmul(
                        pO[:], vtok[pg][:, 2 * m, :], a0[:],
                        start=True, stop=False)
                    # packed a1 lands on the q-cols 128:256 of each par's
                    # window: pO col-blocks 1 and 3 (strided 3D psum out)
                    nc.tensor.matmul(
                        pO3[:, 1::2, :], vtok[pg][:, 2 * m + 1, :], a1[:],
                        start=False, stop=(m == 0), skip_group_check=True)
                    if m > 0:
                        nc.tensor.matmul(
                            pO[:, 0:BLK], spad[2 * pg][:], qT[pg][:, qsl],
                            start=False, stop=False)
                        nc.tensor.matmul(
                            pO[:, BLK:2 * BLK], spad[2 * pg + 1][:], qT[pg][:, qsl],
                            start=False, stop=True)
                    for par in range(2):
                        hr = slice(par * 64, (par + 1) * 64)
                        nc.scalar.activation(
                            oT[pg][hr, qsl],
                            pO[hr, par * BLK:par * BLK + BLK], AF.Identity)

                for pg in range(2):
                    pU = psU.tile([128, 128], F32, name="pU", tag="pu")
                    nc.tensor.matmul(
                        pU[:], ktok[pg][:, 2 * m, :], vtok[pg][:, 2 * m, :],
                        start=True, stop=False)
                    nc.tensor.matmul(
                        pU[:], ktok[pg][:, 2 * m + 1, :], vtok[pg][:, 2 * m + 1, :],
                        start=False, stop=True)
                    for par in range(2):
                        h = 2 * pg + par
                        hr = slice(par * 64, (par + 1) * 64)
                        nc.vector.tensor_tensor(
                            spad[h][hr, hr], pU[hr, hr],
                            spad[h][hr, hr], mybir.AluOpType.add)

            # output staging: the 4 dout-chunks of a (qt, half) accumulate in
            # one [128, 4, 512] tile, shipped with a single DMA per half so
            # the transfer overlaps the other half's compute.
            def proj_tile(qt, half):
                fso = wp.tile([128, 4, 512], BF16, name="fso", tag="fso", bufs=2)
                dcr = range(0, ND // 2) if half == 0 else range(ND // 2, ND)
                for dc in dcr:
                    pf = [psA.tile([128, 512], F32, name="pf", tag="pa"),
                          psU.tile([128, 512], F32, name="pf2", tag="pu"),
                          psO.tile([128, 512], F32, name="pf3", tag="po")][dc % 3]
                    for g2 in range(2):
                        nc.tensor.matmul(
                            pf[:],
                            w2_sb[:, g2, dc * 128:(dc + 1) * 128],
                            oT[g2][:, qt * 512:(qt + 1) * 512],
                            start=(g2 == 0), stop=(g2 == 1))
                    if dc % 2 == 0:
                        nc.vector.tensor_copy(fso[:, dc % 4, :], pf[:])
                    else:
                        nc.scalar.activation(fso[:, dc % 4, :], pf[:], AF.Identity)
                dma_eng = nc.gpsimd if (2 * qt + half) % 2 == 0 else nc.sync
                dma_eng.dma_start(
                    out=out3[:, half * 4:(half + 1) * 4, qt * 512:(qt + 1) * 512],
                    in_=fso[:])

            # final 512-token window in 256-token pieces: block 6's piece can
            # run before block 7's chains, halving the post-chains tail
            def proj_blk(blk):
                fso = wp.tile([128, ND, BLK], BF16, name="fs2", tag="fs2", bufs=2)
                for dc in range(ND):
                    pf = [psA.tile([128, BLK], F32, name="pg1", tag="pa"),
                          psU.tile([128, BLK], F32, name="pg2", tag="pu"),
                          psO.tile([128, BLK], F32, name="pg3", tag="po")][dc % 3]
                    for g2 in range(2):
                        nc.tensor.matmul(
                            pf[:],
                            w2_sb[:, g2, dc * 128:(dc + 1) * 128],
                            oT[g2][:, blk * BLK:(blk + 1) * BLK],
                            start=(g2 == 0), stop=(g2 == 1))
                    if dc % 2 == 0:
                        nc.vector.tensor_copy(fso[:, dc, :], pf[:])
                    else:
                        nc.scalar.activation(fso[:, dc, :], pf[:], AF.Identity)
                dma_eng = nc.gpsimd if blk % 2 == 0 else nc.sync
                dma_eng.dma_start(
                    out=out3[:, :, blk * BLK:(blk + 1) * BLK], in_=fso[:])

            # proj_tile(qt) is emitted one-plus blocks after the chains that
            # produce its oT inputs, so the PE never waits on the Scalar
            # engine's oT copybacks.
            dplan = {3: [(0, 0)], 4: [(0, 1)], 5: [(1, 0)], 6: [(1, 1)],
                     7: [(2, 0), (2, 1)]}
            scores_block(0)
            for m in range(1, NBLK):
                scores_block(m)
                chains_block(m - 1)
                for pt in dplan.get(m, []):
                    proj_tile(*pt)
                if m == NBLK - 1:
                    proj_blk(6)
            chains_block(NBLK - 1)
            proj_blk(7)

    nc.compile()
    return nc


_NC = None


def _get_nc():
    global _NC
    if _NC is None:
        _NC = _build()
    return _NC


def make_core_inputs(x, W1, b1, W2, b2):
    """Shard full inputs into the 8 per-core input dicts."""
    import ml_dtypes
    BF = ml_dtypes.bfloat16
    x = np.asarray(x, dtype=np.float32)
    W1 = np.asarray(W1, dtype=np.float32).astype(BF)
    b1 = np.asarray(b1, dtype=np.float32)
    W2 = np.asarray(W2, dtype=np.float32)

    p = np.arange(128)[:, None]
    f = np.arange(BLK)[None, :]
    mask0 = (f >= p).astype(np.float32)
    mask1 = (f >= p + 128).astype(np.float32)
    masks = np.concatenate([mask0, mask1], axis=1)

    in_maps = []
    for c in range(8):
        b = c // 4
        g = c % 4
        ksl = slice(g * FH, (g + 1) * FH)
        qsl = slice(D + g * FH, D + (g + 1) * FH)
        vsl = slice(2 * D + g * FH, 2 * D + (g + 1) * FH)
        # w1 packed K | V | Q along features (matches f-tile order k0 k1 v0 v1 q0 q1)
        w1p = np.concatenate([W1[:, ksl], W1[:, vsl], W1[:, qsl]], axis=1)
        b1loc = np.concatenate([b1[ksl], b1[vsl], b1[qsl]])
        b1p = np.ascontiguousarray(b1loc.reshape(6, 128).T.astype(np.float32))
        in_maps.append({
            "xT": np.ascontiguousarray(x[b].T.astype(BF)),
            "w1": np.ascontiguousarray(w1p),
            "w2": np.ascontiguousarray(W2[ksl, :].astype(BF)),
            "b1p": b1p,
            "masks": masks,
        })
    return in_maps


def kernel(x, W1, b1, W2, b2):
    nc = _get_nc()
    in_maps = make_core_inputs(x, W1, b1, W2, b2)
    kwargs = {}
    if TRACE:
        kwargs = {"trace": True, "tmpdir": TRACE_DIR}
    res = run_bass_kernel_spmd(nc, in_maps, list(range(8)), **kwargs)
    LAST_RESULTS[0] = res
    b2 = np.asarray(b2, dtype=np.float32)
    out = np.zeros((B, T, D), np.float32)
    for c in range(8):
        oc = np.asarray(res.results[c]["out"]).astype(np.float32)
        oc = oc.reshape(128, ND, T).transpose(1, 0, 2).reshape(D, T)
        out[c // 4] += oc.T
    out += b2[None, None, :]
    return out
